# revision 1
# baseline (speedup 1.0000x reference)
"""Trainium2 Bass kernel for nn_CudaFastWeightSumPerformerLayer.

Performer FAVOR+ fast-weight (causal linear attention) layer.
Sharding: 8 cores = 4 batches x 2 head-groups (4 heads each). Each core
computes qkv projection, prime features, the chunked causal linear-attention
scan, and its partial w_o projection on device. Host sums the two partials
per batch, adds the residual, and applies the final LayerNorm (v1).

Math restructure (validated vs reference to ~2e-4 rel err):
  - The FAVOR+ diag term exp(-0.5|x|^2) cancels in the normalized output,
    so features are just [exp(d), exp(-d)], d = (x * dh^-0.25) @ proj.
  - kp normalization (1/sum) is folded into V' columns; the attention
    denominator and the q-feature sum arrive as extra output columns via
    ones-columns in V' and the scan state.
  - out_final = out_raw / (denom_raw + eps * qsum).
Chunked scan (chunk=128): B[j,t] = kp_j . qp_t (masked j<=t),
  out_c = B^T @ V' + qp_c @ S;  S += kp_c^T @ V'.
Features and scan matmuls run in bf16 (PSUM accumulation fp32); qkv/prime
and the w_o projection run in fp32r (full-rate fp32 storage).
"""

import numpy as np

L, DM, DH, M = 2048, 512, 64, 256
F = 2 * M          # 512 feature dim
NH = 8             # total heads
HPC = 4            # heads per core
B = 4
CH = 128           # scan chunk
NCH = L // CH      # 16
SCALE = DH ** -0.5
EPS_ATTN = 1e-5
EPS_LN = 1e-5
N_CORES = 8

_CACHE = {}


def _build_nc():
    import concourse.bacc as bacc
    import concourse.tile as tile
    from concourse import mybir

    f32 = mybir.dt.float32
    f32r = mybir.dt.float32r
    bf16 = mybir.dt.bfloat16
    AF = mybir.ActivationFunctionType
    ALU = mybir.AluOpType

    nc = bacc.Bacc("TRN2", target_bir_lowering=False, debug=False,
                   num_devices=N_CORES)

    hT_d = nc.dram_tensor("hT", [DM, L], f32r, kind="ExternalInput")
    wqkT_d = nc.dram_tensor("wqkT", [DM, 128 * HPC], f32r, kind="ExternalInput")
    wvT_d = nc.dram_tensor("wvT", [DM, 64 * HPC], f32r, kind="ExternalInput")
    projs2_d = nc.dram_tensor("projs2", [128, M], f32r, kind="ExternalInput")
    woT_d = nc.dram_tensor("woT", [64 * HPC, DM], f32r, kind="ExternalInput")
    identmask_d = nc.dram_tensor("identmask", [128, 256], f32,
                                 kind="ExternalInput")
    part_d = nc.dram_tensor("part", [L, DM], f32, kind="ExternalOutput")

    with tile.TileContext(nc) as tc:
        from contextlib import ExitStack
        with ExitStack() as ctx:
            consts = ctx.enter_context(tc.tile_pool(name="consts", bufs=1))
            qkpool = ctx.enter_context(tc.tile_pool(name="qkpool", bufs=1))
            vpool = ctx.enter_context(tc.tile_pool(name="vpool", bufs=1))
            onorm = ctx.enter_context(tc.tile_pool(name="onorm", bufs=1))

            # ---- constant loads (batched: one DMA per DRAM tensor) ----
            wqk_all = consts.tile([128, 4, 128 * HPC], f32r, tag="wqka",
                                  name="wqka")
            wv_all = consts.tile([128, 4, 64 * HPC], f32r, tag="wva",
                                 name="wva")
            wo_all = consts.tile([128, 2, DM], f32r, tag="woa", name="woa")
            # proj duplicated on both partition halves so q (parts 0-63) and
            # k (parts 64-127) prime matmuls both find lhsT at their base
            proj_sb = consts.tile([128, M], f32r, tag="proj", name="proj")
            im_sb = consts.tile([128, 256], f32, tag="im", name="im")
            ones_sb = consts.tile([128, 1], bf16, tag="ones", name="ones")
            nc.sync.dma_start(
                out=wqk_all,
                in_=wqkT_d[:].rearrange("(k p) n -> p k n", p=128))
            nc.sync.dma_start(
                out=wv_all,
                in_=wvT_d[:].rearrange("(k p) n -> p k n", p=128))
            nc.sync.dma_start(
                out=wo_all,
                in_=woT_d[:].rearrange("(k p) n -> p k n", p=128))
            nc.sync.dma_start(out=proj_sb, in_=projs2_d[:, :])
            nc.sync.dma_start(out=im_sb, in_=identmask_d[:, :])
            nc.vector.memset(ones_sb, 1.0)
            wqk_sb = [wqk_all[:, kc, :] for kc in range(4)]
            wv_sb = [wv_all[:, kc, :] for kc in range(4)]
            wo_sb = [wo_all[:, jb, :] for jb in range(2)]
            ident_sb = im_sb[:, 0:128]
            masku_sb = im_sb[:, 128:256]

            # persistent outputs of the scan, token-major (t, 4 heads x 64)
            on_tc = [onorm.tile([128, 64 * HPC], f32, tag=f"on{c}", name=f"on{c}")
                     for c in range(NCH)]

            # ---- phase 1: qkv projection (fp32r) ----
            qk_sb = [qkpool.tile([128, L], f32r, tag=f"qk{m}", name=f"qk{m}")
                     for m in range(HPC)]
            v_sb = [vpool.tile([128, 64 * HPC], f32, tag=f"v{c}", name=f"v{c}")
                    for c in range(NCH)]
            with tc.tile_pool(name="hTp", bufs=1) as hTp, \
                 tc.tile_pool(name="p1ps", bufs=2, space="PSUM") as p1ps, \
                 tc.tile_pool(name="p1vps", bufs=2, space="PSUM") as p1vps:
                hT_all = hTp.tile([128, 4, L], f32r, tag="hTa", name="hTa")
                nc.sync.dma_start(
                    out=hT_all,
                    in_=hT_d[:].rearrange("(k p) t -> p k t", p=128))
                hT_sb = [hT_all[:, kc, :] for kc in range(4)]
                for m in range(HPC):
                    for t4 in range(4):
                        ps = p1ps.tile([128, 512], f32, tag="qkps", name="qkps")
                        for kc in range(4):
                            nc.tensor.matmul(
                                ps[:],
                                wqk_sb[kc][:, 128 * m:128 * (m + 1)],
                                hT_sb[kc][:, 512 * t4:512 * (t4 + 1)],
                                start=(kc == 0), stop=(kc == 3))
                        nc.scalar.copy(
                            out=qk_sb[m][:, 512 * t4:512 * (t4 + 1)], in_=ps[:])
                for c in range(NCH):
                    ps = p1vps.tile([128, 64 * HPC], f32, tag="vps", name="vps")
                    for kc in range(4):
                        nc.tensor.matmul(
                            ps[:],
                            hT_sb[kc][:, 128 * c:128 * (c + 1)],
                            wv_sb[kc][:],
                            start=(kc == 0), stop=(kc == 3))
                    nc.scalar.copy(out=v_sb[c][:], in_=ps[:])

            # ---- phases 2+3: two heads in flight (parity-tagged tiles) ----
            # PSUM budget (8 banks): d_ps 4 (shared), bk 1 (shared),
            # out 1+1 (per parity), delta 1 (shared).
            with tc.tile_pool(name="feat", bufs=1) as featp, \
                 tc.tile_pool(name="misc", bufs=3) as miscp, \
                 tc.tile_pool(name="stsb", bufs=1) as stp, \
                 tc.tile_pool(name="dps", bufs=1, space="PSUM") as dps, \
                 tc.tile_pool(name="bkps", bufs=2, space="PSUM") as bkps, \
                 tc.tile_pool(name="ops", bufs=3, space="PSUM") as ops, \
                 tc.tile_pool(name="stps", bufs=1, space="PSUM") as stps:
                # one delta tile shared by both parities (ping-pong on the bank)
                d_del = stps.tile([128, 66 * 4], f32, tag="sdel", name="sdel")
                for fc in range(4):
                    nc.vector.memset(d_del[:, 66 * fc + 65:66 * fc + 66], 0.0)
                for m in range(HPC):
                    p = m % 2
                    # -- prime features (feature-major qp/kp, bf16) --
                    qp = [featp.tile([128, L], bf16, tag=f"qp{fc}_{p}",
                                     name=f"qp{fc}_{p}") for fc in range(4)]
                    kp = [featp.tile([128, L], bf16, tag=f"kp{fc}_{p}",
                                     name=f"kp{fc}_{p}") for fc in range(4)]
                    for (src_off, dst) in ((0, qp), (64, kp)):
                        for fh in range(2):
                            for t2 in range(2):
                                d_ps = dps.tile([128, 1024], f32, tag="dps",
                                                name="dps")
                                for tt in range(2):
                                    t4 = 2 * t2 + tt
                                    nc.tensor.matmul(
                                        d_ps[:, 512 * tt:512 * (tt + 1)],
                                        proj_sb[src_off:src_off + 64,
                                                128 * fh:128 * (fh + 1)],
                                        qk_sb[m][src_off:src_off + 64,
                                                 512 * t4:512 * (t4 + 1)],
                                        start=True, stop=True)
                                sl2 = slice(1024 * t2, 1024 * (t2 + 1))
                                nc.scalar.activation(out=dst[fh][:, sl2],
                                                     in_=d_ps[:], func=AF.Exp)
                                with nc.allow_low_precision(reason="bf16"):
                                    nc.vector.reciprocal(
                                        out=dst[fh + 2][:, sl2],
                                        in_=dst[fh][:, sl2])
                    # -- token-major k features (for the state update) --
                    # kp_t[t, 512c+f], f in [0,256) = exp(d), [256,512) = 1/exp(d)
                    kp_t = featp.tile([128, 512 * NCH], bf16, tag=f"kpt_{p}",
                                      name=f"kpt_{p}")
                    kp_t_v = kp_t[:].rearrange("p (c f) -> p c f", f=512)
                    for qtr in range(4):
                        dt_ps = dps.tile([128, 1024], f32, tag="dps",
                                         name="dtps")
                        dt_v = dt_ps[:].rearrange("p (c f) -> p c f", f=256)
                        for cc in range(4):
                            c = 4 * qtr + cc
                            nc.tensor.matmul(
                                dt_v[:, cc, :],
                                qk_sb[m][64:128, 128 * c:128 * (c + 1)],
                                proj_sb[64:128, :],
                                start=True, stop=True)
                        nc.scalar.activation(
                            out=kp_t_v[:, 4 * qtr:4 * (qtr + 1), 0:256],
                            in_=dt_v[:, :, :], func=AF.Exp)
                        with nc.allow_low_precision(reason="bf16 features"):
                            nc.vector.reciprocal(
                                out=kp_t_v[:, 4 * qtr:4 * (qtr + 1), 256:512],
                                in_=kp_t_v[:, 4 * qtr:4 * (qtr + 1), 0:256])

                    # -- scan (state in SBUF bf16, updated via delta PSUM) --
                    st_sb = stp.tile([128, 66 * 4], bf16, tag=f"st_{p}",
                                     name=f"st_{p}")
                    nc.vector.memset(st_sb, 0.0)
                    for fc in range(4):
                        nc.vector.memset(st_sb[:, 66 * fc + 65:66 * fc + 66], 1.0)
                    # V' tile reused across chunks; col 65 stays 0 so the
                    # intra matmul's accumulation group can span cols 0:66
                    vp = stp.tile([128, 66], bf16, tag=f"vp_{p}", name=f"vp_{p}")
                    nc.vector.memset(vp[:, 65:66], 0.0)
                    for c in range(NCH):
                        sl = slice(128 * c, 128 * (c + 1))
                        # keys x queries + ksum column, one bank/one group
                        bk = bkps.tile([128, 129], f32, tag="bk", name="bk")
                        for fc in range(4):
                            nc.tensor.matmul(bk[:, 0:128], kp[fc][:, sl],
                                             qp[fc][:, sl],
                                             start=(fc == 0), stop=(fc == 3))
                        for fc in range(4):
                            nc.tensor.matmul(bk[:, 128:129],
                                             kp[fc][:, sl], ones_sb[:],
                                             start=(fc == 0), stop=(fc == 3))
                        bm = miscp.tile([128, 128], bf16, tag=f"bm_{p}",
                                        name=f"bm_{p}")
                        nc.vector.tensor_mul(out=bm[:], in0=bk[:, 0:128],
                                             in1=masku_sb[:])
                        # V' = [v/ksum | 1/ksum | 0] (bf16)
                        rk = miscp.tile([128, 1], f32, tag=f"rk_{p}",
                                        name=f"rk_{p}")
                        nc.vector.reciprocal(out=rk[:], in_=bk[:, 128:129])
                        nc.gpsimd.tensor_scalar_mul(
                            out=vp[:, 0:64],
                            in0=v_sb[c][:, 64 * m:64 * (m + 1)],
                            scalar1=rk[:])
                        nc.gpsimd.tensor_copy(out=vp[:, 64:65], in_=rk[:])
                        # out_c = B^T @ V' (intra) + qp_c @ S (inter + qsum)
                        o_ps = ops.tile([128, 66], f32, tag="o", name="o")
                        nc.tensor.matmul(o_ps[:], bm[:], vp[:],
                                         start=True, stop=False)
                        for fc in range(4):
                            nc.tensor.matmul(
                                o_ps[:],
                                qp[fc][:, sl],
                                st_sb[:, 66 * fc:66 * fc + 66],
                                start=False, stop=(fc == 3))
                        # state update: delta = kp_c^T @ V' per fc block
                        # (4 closed psum groups in one bank), then bf16 add
                        for fc in range(4):
                            nc.tensor.matmul(
                                d_del[:, 66 * fc:66 * fc + 65],
                                kp_t_v[:, c, 128 * fc:128 * (fc + 1)],
                                vp[:, 0:65],
                                start=True, stop=True)
                        with nc.allow_low_precision(reason="bf16 state"):
                            nc.vector.tensor_add(out=st_sb[:], in0=st_sb[:],
                                                 in1=d_del[:])
                        # normalize: out / (denom + eps*qsum)
                        rcp = miscp.tile([128, 1], f32, tag=f"rcp_{p}",
                                         name=f"rcp_{p}")
                        nc.vector.tensor_scalar(
                            out=rcp[:], in0=o_ps[:, 65:66],
                            scalar1=EPS_ATTN, scalar2=o_ps[:, 64:65],
                            op0=ALU.mult, op1=ALU.add)
                        nc.vector.reciprocal(out=rcp[:], in_=rcp[:])
                        nc.vector.tensor_scalar_mul(
                            out=on_tc[c][:, 64 * m:64 * (m + 1)],
                            in0=o_ps[:, 0:64],
                            scalar1=rcp[:])

            # ---- phase 4: transpose out_norm -> (j, t) ----
            with tc.tile_pool(name="onT", bufs=1) as onTp, \
                 tc.tile_pool(name="atsb", bufs=3) as atsbp, \
                 tc.tile_pool(name="trps", bufs=2, space="PSUM") as trps, \
                 tc.tile_pool(name="atps", bufs=2, space="PSUM") as atps:
                onT_sb = [onTp.tile([128, L], f32r, tag=f"onT{jb}", name=f"onT{jb}")
                          for jb in range(2)]
                for c in range(NCH):
                    for jb in range(2):
                        t_ps = trps.tile([128, 128], f32, tag="trps", name="trps")
                        nc.tensor.transpose(
                            t_ps[:], on_tc[c][:, 128 * jb:128 * (jb + 1)],
                            ident_sb[:])
                        nc.scalar.copy(out=onT_sb[jb][:, 128 * c:128 * (c + 1)],
                                       in_=t_ps[:])
                # ---- phase 5: partial attn = out_norm @ woT ----
                # stage 4 chunks per SBUF tile -> 4 batched output DMAs
                part_v = part_d[:].rearrange("(c p) d -> p c d", p=128)
                for g in range(4):
                    a_sb = atsbp.tile([128, 4, DM], f32, tag="atsb",
                                      name="atsb")
                    for cc in range(4):
                        c = 4 * g + cc
                        a_ps = atps.tile([128, DM], f32, tag="atps",
                                         name="atps")
                        for jb in range(2):
                            nc.tensor.matmul(
                                a_ps[:],
                                onT_sb[jb][:, 128 * c:128 * (c + 1)],
                                wo_sb[jb][:],
                                start=(jb == 0), stop=(jb == 1))
                        if cc % 2 == 0:
                            nc.scalar.copy(out=a_sb[:, cc, :], in_=a_ps[:])
                        else:
                            nc.vector.tensor_copy(out=a_sb[:, cc, :],
                                                  in_=a_ps[:])
                    nc.sync.dma_start(out=part_v[:, 4 * g:4 * (g + 1), :],
                                      in_=a_sb[:])

    nc.compile()
    return nc


def _host_prep(h, w_qkv, w_o, proj_matrix):
    """Build per-core input maps."""
    projs = (proj_matrix * (DH ** -0.25)).astype(np.float32)
    projs2 = np.concatenate([projs, projs], axis=0)  # (128, M), both halves
    ident = np.eye(128, dtype=np.float32)
    masku = (np.arange(128)[:, None] <= np.arange(128)[None, :]).astype(np.float32)
    identmask = np.concatenate([ident, masku], axis=1)  # (128, 256)
    woT_full = (w_o.T * SCALE).astype(np.float32)  # (H*DH, DM)

    in_maps = []
    for core in range(N_CORES):
        b, hg = core // 2, core % 2
        heads = [HPC * hg + m for m in range(HPC)]
        hT = np.ascontiguousarray(h[:, b, :].T)
        wqkT = np.empty((DM, 128 * HPC), np.float32)
        wvT = np.empty((DM, 64 * HPC), np.float32)
        woT = np.empty((64 * HPC, DM), np.float32)
        for m, hh in enumerate(heads):
            blk = w_qkv[192 * hh:192 * (hh + 1)]  # (192, DM) = [q64,k64,v64]
            wqkT[:, 128 * m:128 * m + 64] = blk[0:64].T
            wqkT[:, 128 * m + 64:128 * (m + 1)] = blk[64:128].T
            wvT[:, 64 * m:64 * (m + 1)] = blk[128:192].T
            woT[64 * m:64 * (m + 1), :] = woT_full[64 * hh:64 * (hh + 1), :]
        in_maps.append({
            "hT": hT, "wqkT": wqkT, "wvT": wvT, "projs2": projs2,
            "woT": woT, "identmask": identmask,
        })
    return in_maps


def kernel(h, w_qkv, w_o, ln_gamma, ln_beta, proj_matrix):
    from concourse.bass_utils import run_bass_kernel_spmd

    h = np.asarray(h, np.float32)
    w_qkv = np.asarray(w_qkv, np.float32)
    w_o = np.asarray(w_o, np.float32)
    ln_gamma = np.asarray(ln_gamma, np.float32)
    ln_beta = np.asarray(ln_beta, np.float32)
    proj_matrix = np.asarray(proj_matrix, np.float32)

    if "nc" not in _CACHE:
        _CACHE["nc"] = _build_nc()
    nc = _CACHE["nc"]

    in_maps = _host_prep(h, w_qkv, w_o, proj_matrix)
    res = run_bass_kernel_spmd(nc, in_maps, core_ids=list(range(N_CORES)))

    out = np.empty((L, B, DM), np.float32)
    for b in range(B):
        attn = res.results[2 * b]["part"] + res.results[2 * b + 1]["part"]
        x = h[:, b, :] + attn
        mu = x.mean(-1, keepdims=True)
        var = ((x - mu) ** 2).mean(-1, keepdims=True)
        out[:, b, :] = (x - mu) / np.sqrt(var + EPS_LN) * ln_gamma + ln_beta
    return out

